# revision 3
# baseline (speedup 1.0000x reference)
"""Multi-head attention Trainium2 Bass kernel (nn_MultiHeadAttention_69655779607087).

Problem (hardcoded): B=4, L=2048, D_MODEL=1024, H=16, D_QK=D_V=64, fp32.

Sharding v2 (8 cores, tensor-parallel heads x batch, host-side pair reduce):
core c handles batch b=c//2 and head-half hh=c%2 (8 heads), producing a
PARTIAL output [2048, 1024] (its heads' contribution to the out-projection).
The host sums the two partial outputs per batch (the "all-reduce" of the
sharding hint, done on host during unshard) and stacks the 4 batches.

Per-core dataflow (bf16 matmul operands, fp32 PSUM):
  proj     QT/KT[hd,2048] per head-pair frame (head 2p on partitions 0-63,
           head 2p+1 on 64-127), VA[tok,head,64] token-major; X^T for K/V
           kept SBUF-resident, xq streamed. Projections for pair p+1 are
           interleaved into pair p's attention loop to fill the PE while
           the Scalar engine (exp) is the bottleneck.
  scores   row-tiled pair: two concurrent K=64 matmuls (head A rows 0-63,
           head B rows 64-127) -> separate PSUM tiles, no zero-padding.
  softmax  one exp per head per s-block [128,1024] (ScalarE), denominators
           via M=1 ones-matmuls col-tiled to PSUM rows 0/32, accumulated
           on DVE into DEN[33,1024] (only rows 0,32 meaningful).
  attnV    col-tiled pair: head A (M=64) -> OP rows 0-63, head B -> rows
           64-127, accumulated over 16 s-blocks in one [128,1024] bank-pair.
  norm     OP copied unnormalized to HT (bf16); reciprocal via DVE
           reciprocal_approx_fast, broadcast via DRAM bounce, one DVE mul.
  out-proj out[tok,dm] partial = HT^T(lhsT) @ Wout, streamed to DRAM.
"""

import os
import sys

for _p in ("/opt/trn_rl_repo", "/opt/pypackages"):
    if _p not in sys.path:
        sys.path.append(_p)

import numpy as np

H_TOT, D, DK, DV = 16, 1024, 64, 64
B, L = 4, 2048
H = 8  # heads per core
NP = 4  # head pairs per core
P = 128
NKB = D // P  # 8 contraction blocks over d_model
NSB = L // P  # 16 key-token blocks
NQG = 2  # query groups of 1024

_CACHE = {}


def _build_bass():
    import concourse.bass as bass
    import concourse.tile as tile
    from concourse import mybir
    from concourse.bass import ts

    f32 = mybir.dt.float32
    bf16 = mybir.dt.bfloat16
    EXP = mybir.ActivationFunctionType.Exp

    nc = bass.Bass()
    xqt = nc.dram_tensor("xqt", [D, L], bf16, kind="ExternalInput")
    xkt = nc.dram_tensor("xkt", [D, L], bf16, kind="ExternalInput")
    xvt = nc.dram_tensor("xvt", [D, L], bf16, kind="ExternalInput")
    wq = nc.dram_tensor("wq", [NKB, P, NP, P], bf16, kind="ExternalInput")
    wk = nc.dram_tensor("wk", [NKB, P, NP, P], bf16, kind="ExternalInput")
    wv = nc.dram_tensor("wv", [D, H * DV], bf16, kind="ExternalInput")
    wout = nc.dram_tensor("wout", [H * DV, D], bf16, kind="ExternalInput")
    out = nc.dram_tensor("out", [L, D], f32, kind="ExternalOutput")

    lp = nc.allow_low_precision(
        reason="bf16 matmul operands; accumulation stays fp32 in PSUM"
    )
    lp.__enter__()
    with tile.TileContext(nc) as tc:
        with (
            tc.tile_pool(name="persist", bufs=1) as persist,
            tc.tile_pool(name="xin", bufs=3) as xin,
            tc.tile_pool(name="aep", bufs=4) as aep,
            tc.tile_pool(name="bcp", bufs=2) as bcp,
            tc.tile_pool(name="outp", bufs=2) as outp,
            tc.tile_pool(name="dramp", bufs=2, space="DRAM") as dramp,
            tc.tile_pool(name="ps", bufs=1, space="PSUM") as ps,
        ):
            # ---- persistent SBUF ----
            XK = persist.tile([P, NKB, L], bf16)  # 32 KB/part
            XV = persist.tile([P, NKB, L], bf16)  # 32 KB/part
            QT = persist.tile([P, NP, L], bf16)  # 16 KB/part
            KT = persist.tile([P, NP, L], bf16)  # 16 KB/part
            VA = persist.tile([P, NSB, H, DV], bf16)  # 16 KB/part
            HT = persist.tile([P, NP, L], bf16)  # 16 KB/part
            WQ = persist.tile([P, NKB, NP, P], bf16)  # 8 KB/part
            WK = persist.tile([P, NKB, NP, P], bf16)  # 8 KB/part
            WV = persist.tile([P, NKB, H * DV], bf16)  # 8 KB/part
            WO = persist.tile([P, NP, D], bf16)  # 8 KB/part
            ONES = persist.tile([P, 2], bf16)
            DEN = persist.tile([33, 1024], f32)  # rows 0,32 meaningful
            REC = persist.tile([33, 1024], f32)
            REC16 = persist.tile([33, 1024], bf16)

            # ---- prologue DMAs ----
            for k in range(NKB):
                nc.sync.dma_start(out=WQ[:, k], in_=wq[k])
                nc.sync.dma_start(out=WK[:, k], in_=wk[k])
                nc.sync.dma_start(out=WV[:, k], in_=wv[ts(k, P), :])
            for k in range(NP):
                nc.sync.dma_start(out=WO[:, k], in_=wout[ts(k, P), :])
            nc.gpsimd.memset(ONES[:, :], 1.0)
            for k in range(NKB):
                nc.sync.dma_start(out=XK[:, k], in_=xkt[ts(k, P), :])
                nc.gpsimd.dma_start(out=XV[:, k], in_=xvt[ts(k, P), :])

            # ---- helper emitters ----
            def emit_k_proj_group(p, g):
                # KT[:, p, g*1024:+1024] from resident XK
                pt = ps.tile([P, 1024], f32, tag="sp", bufs=3, name=f"kp_{p}_{g}")
                for k in range(NKB):
                    for qn in range(2):
                        c0 = g * 1024 + qn * 512
                        nc.tensor.matmul(
                            pt[:, qn * 512 : qn * 512 + 512],
                            lhsT=WK[:, k, p, :],
                            rhs=XK[:, k, c0 : c0 + 512],
                            start=(k == 0),
                            stop=(k == NKB - 1),
                        )
                nc.vector.tensor_copy(KT[:, p, g * 1024 : g * 1024 + 1024], pt)

            def emit_q_proj_group(p, g):
                pt = ps.tile([P, 1024], f32, tag="sp", bufs=3, name=f"qp_{p}_{g}")
                for k in range(NKB):
                    xt = xin.tile([P, 1024], bf16, tag="xq")
                    nc.gpsimd.dma_start(
                        out=xt, in_=xqt[ts(k, P), g * 1024 : g * 1024 + 1024]
                    )
                    for qn in range(2):
                        nc.tensor.matmul(
                            pt[:, qn * 512 : qn * 512 + 512],
                            lhsT=WQ[:, k, p, :],
                            rhs=xt[:, qn * 512 : qn * 512 + 512],
                            start=(k == 0),
                            stop=(k == NKB - 1),
                        )
                nc.vector.tensor_copy(QT[:, p, g * 1024 : g * 1024 + 1024], pt)

            def emit_v_proj_pair(j):
                # token blocks 2j, 2j+1 -> VA
                pt = ps.tile([P, 1024], f32, tag="sp", bufs=3, name=f"vp_{j}")
                for k in range(NKB):
                    for jj in range(2):
                        nc.tensor.matmul(
                            pt[:, jj * 512 : jj * 512 + 512],
                            lhsT=XV[:, k, ts(2 * j + jj, P)],
                            rhs=WV[:, k, :],
                            start=(k == 0),
                            stop=(k == NKB - 1),
                        )
                for jj in range(2):
                    nc.vector.tensor_copy(
                        VA[:, 2 * j + jj, :, :],
                        pt[:, jj * 512 : jj * 512 + 512].rearrange(
                            "p (h v) -> p h v", h=H
                        ),
                    )

            # ---- prologue compute: pair 0 Q/K, V blocks 0-7 ----
            for g in range(NQG):
                emit_k_proj_group(0, g)
            for g in range(NQG):
                emit_q_proj_group(0, g)
            for j in range(4):
                emit_v_proj_pair(j)

            # ---- attention + interleaved projections ----
            for p in range(NP):
                hA, hB = 2 * p, 2 * p + 1
                for qg in range(NQG):
                    # deferred work emitted inside the s loop, keyed by s
                    defer = {}
                    if p == 0 and qg == 0:
                        for s_at, j in zip((1, 3, 5, 7), (4, 5, 6, 7)):
                            defer.setdefault(s_at, []).append(
                                lambda j=j: emit_v_proj_pair(j)
                            )
                    if p < NP - 1:
                        which = (
                            ((emit_k_proj_group, 0), (emit_k_proj_group, 1))
                            if qg == 0
                            else ((emit_q_proj_group, 0), (emit_q_proj_group, 1))
                        )
                        for s_at, (fn, g) in zip((5, 11), which):
                            defer.setdefault(s_at, []).append(
                                lambda fn=fn, g=g: fn(p + 1, g)
                            )

                    op = ps.tile([P, 1024], f32, tag="op", bufs=1, name=f"op_{p}_{qg}")
                    for s in range(NSB):
                        sa = ps.tile([P, 1024], f32, tag="sp", bufs=3)
                        sb = ps.tile([P, 1024], f32, tag="sp", bufs=3)
                        for qn in range(2):
                            c0 = qg * 1024 + qn * 512
                            nc.tensor.matmul(
                                sa[:, qn * 512 : qn * 512 + 512],
                                lhsT=KT[0:64, p, ts(s, P)],
                                rhs=QT[0:64, p, c0 : c0 + 512],
                                start=True,
                                stop=True,
                            )
                            nc.tensor.matmul(
                                sb[:, qn * 512 : qn * 512 + 512],
                                lhsT=KT[64:128, p, ts(s, P)],
                                rhs=QT[64:128, p, c0 : c0 + 512],
                                start=True,
                                stop=True,
                            )
                        aea = aep.tile([P, 1024], bf16, tag="ae")
                        aeb = aep.tile([P, 1024], bf16, tag="ae")
                        nc.scalar.activation(out=aea, in_=sa, func=EXP, scale=0.125)
                        nc.scalar.activation(out=aeb, in_=sb, func=EXP, scale=0.125)
                        # interleaved projection work (independent of exp)
                        for fn in defer.get(s, ()):
                            fn()
                        for qn in range(2):
                            nc.tensor.matmul(
                                op[0:64, qn * 512 : qn * 512 + 512],
                                lhsT=VA[:, s, hA, :],
                                rhs=aea[:, qn * 512 : qn * 512 + 512],
                                start=(s == 0),
                                stop=(s == NSB - 1),
                            )
                            nc.tensor.matmul(
                                op[64:128, qn * 512 : qn * 512 + 512],
                                lhsT=VA[:, s, hB, :],
                                rhs=aeb[:, qn * 512 : qn * 512 + 512],
                                start=(s == 0),
                                stop=(s == NSB - 1),
                            )
                        dn = ps.tile([P, 1024], f32, tag="sp", bufs=3)
                        for qn in range(2):
                            nc.tensor.matmul(
                                dn[0:1, qn * 512 : qn * 512 + 512],
                                lhsT=ONES[:, 0:1],
                                rhs=aea[:, qn * 512 : qn * 512 + 512],
                                start=True,
                                stop=True,
                            )
                            nc.tensor.matmul(
                                dn[32:33, qn * 512 : qn * 512 + 512],
                                lhsT=ONES[:, 1:2],
                                rhs=aeb[:, qn * 512 : qn * 512 + 512],
                                start=True,
                                stop=True,
                            )
                        if s == 0:
                            nc.vector.tensor_copy(DEN[:, :], dn[0:33, :])
                        else:
                            nc.vector.tensor_add(DEN[:, :], DEN[:, :], dn[0:33, :])

                    # ---- qg epilogue: stash heads, reciprocal, broadcast, scale
                    cols = slice(qg * 1024, qg * 1024 + 1024)
                    nc.vector.tensor_copy(HT[:, p, cols], op)
                    nc.vector.reciprocal(REC[:, :], DEN[:, :])
                    nc.vector.tensor_copy(REC16[:, :], REC[:, :])
                    rcb = dramp.tile([2, 1024], bf16, tag="rcb", name=f"rcb_{p}_{qg}")
                    nc.sync.dma_start(out=rcb[0:1, :], in_=REC16[0:1, :])
                    nc.sync.dma_start(out=rcb[1:2, :], in_=REC16[32:33, :])
                    bc = bcp.tile([P, 1024], bf16, tag="bc")
                    nc.gpsimd.dma_start(
                        out=bc[0:64, :], in_=rcb[0:1, :].to_broadcast((64, 1024))
                    )
                    nc.sync.dma_start(
                        out=bc[64:128, :], in_=rcb[1:2, :].to_broadcast((64, 1024))
                    )
                    nc.vector.tensor_mul(HT[:, p, cols], HT[:, p, cols], bc[:, :])

            # ---- out-projection (partial over this core's heads) ----
            for m in range(NSB):
                pt = ps.tile([P, 1024], f32, tag="sp", bufs=3, name=f"po_{m}")
                for dh in range(2):
                    for kp in range(NP):
                        nc.tensor.matmul(
                            pt[:, dh * 512 : dh * 512 + 512],
                            lhsT=HT[:, kp, ts(m, P)],
                            rhs=WO[:, kp, dh * 512 : dh * 512 + 512],
                            start=(kp == 0),
                            stop=(kp == NP - 1),
                        )
                ot = outp.tile([P, 1024], f32, tag="ot", name=f"ot_{m}")
                nc.vector.tensor_copy(ot, pt)
                (nc.gpsimd if m % 2 == 0 else nc.sync).dma_start(
                    out=out[ts(m, P), :], in_=ot
                )
    lp.__exit__(None, None, None)

    _split_multi_waits(nc)
    return nc


def _split_multi_waits(nc, max_waits: int = 1):
    """Walrus's setupSyncWait rejects instructions carrying more than a
    struct-specific number of sync waits (e.g. the Tile kernel-tail Drain
    gathers one wait per live semaphore). Hoist excess waits into prepended
    single-wait NoOps on the same engine."""
    from concourse import mybir

    for f in nc.m.functions:
        for blk in f.blocks:
            out = []
            for inst in blk.instructions:
                si = inst.sync_info
                waits = list(si.on_wait) if (si is not None and si.on_wait) else []
                if len(waits) > max_waits:
                    keep = waits[-max_waits:]
                    for w in waits[:-max_waits]:
                        nop = mybir.InstNoOp(
                            name=nc.get_next_instruction_name(),
                            ins=[],
                            outs=[],
                            sync_info=mybir.SyncInfo(on_wait=[w], on_update=[]),
                        )
                        nop.engine = inst.engine
                        try:
                            nop.bass_nofuse = True
                        except Exception:
                            pass
                        nc.register_instruction(nop)
                        out.append(nop)
                    si.on_wait = keep
                out.append(inst)
            blk.instructions = out


def _get_nc():
    if "nc" not in _CACHE:
        _CACHE["nc"] = _build_bass()
    return _CACHE["nc"]


def _prep_in_maps(x_query, x_key, x_value, Wq, Wk, Wv, Wout):
    import ml_dtypes

    bf = ml_dtypes.bfloat16
    x_query = np.asarray(x_query, dtype=np.float32)
    x_key = np.asarray(x_key, dtype=np.float32)
    x_value = np.asarray(x_value, dtype=np.float32)
    Wq = np.asarray(Wq, np.float32)
    Wk = np.asarray(Wk, np.float32)
    Wv = np.asarray(Wv, np.float32)
    Wout = np.asarray(Wout, np.float32)

    # per-batch transposed activations (shared by the 2 cores of a batch)
    xqT = [np.ascontiguousarray(x_query[b].T).astype(bf) for b in range(B)]
    xkT = [np.ascontiguousarray(x_key[b].T).astype(bf) for b in range(B)]
    xvT = [np.ascontiguousarray(x_value[b].T).astype(bf) for b in range(B)]

    # per head-half weight slices
    wq_h, wk_h, wv_h, wo_h = [], [], [], []
    for hh in range(2):
        hs = slice(hh * H, hh * H + H)
        # [H,D,dk] -> [D, H*dk] -> [k,128,NP,128]
        wq_cat = Wq[hs].transpose(1, 0, 2).reshape(D, H * DK)
        wk_cat = Wk[hs].transpose(1, 0, 2).reshape(D, H * DK)
        wq_h.append(np.ascontiguousarray(wq_cat.reshape(NKB, P, NP, P)).astype(bf))
        wk_h.append(np.ascontiguousarray(wk_cat.reshape(NKB, P, NP, P)).astype(bf))
        wv_h.append(
            np.ascontiguousarray(
                Wv[hs].transpose(1, 0, 2).reshape(D, H * DV)
            ).astype(bf)
        )
        wo_h.append(
            np.ascontiguousarray(Wout[hh * H * DV : (hh + 1) * H * DV, :]).astype(bf)
        )

    in_maps = []
    for c in range(8):
        b, hh = divmod(c, 2)
        in_maps.append(
            {
                "xqt": xqT[b],
                "xkt": xkT[b],
                "xvt": xvT[b],
                "wq": wq_h[hh],
                "wk": wk_h[hh],
                "wv": wv_h[hh],
                "wout": wo_h[hh],
            }
        )
    return in_maps


def kernel(x_query, x_key, x_value, Wq, Wk, Wv, Wout):
    from concourse.bass_utils import run_bass_kernel_spmd

    nc = _get_nc()
    in_maps = _prep_in_maps(x_query, x_key, x_value, Wq, Wk, Wv, Wout)
    trace = bool(int(os.environ.get("MHA_TRACE", "0")))
    res = run_bass_kernel_spmd(nc, in_maps, list(range(8)), trace=trace)
    _CACHE["last_result"] = res
    out = np.empty((B, L, D), np.float32)
    for b in range(B):
        out[b] = res.results[2 * b]["out"] + res.results[2 * b + 1]["out"]
    return out


# revision 8
# speedup vs baseline: 1.1548x; 1.1548x over previous
"""Multi-head attention Trainium2 Bass kernel (nn_MultiHeadAttention_69655779607087).

Problem (hardcoded): B=4, L=2048, D_MODEL=1024, H=16, D_QK=D_V=64, fp32.

Sharding (8 cores, tensor-parallel heads x batch, host-side pair reduce):
core c handles batch b=c//2 and head-half hh=c%2 (8 heads), producing a
PARTIAL output [2048, 1024] (its heads' contribution to the out-projection).
The host sums the two partial outputs per batch (the "all-reduce" of the
sharding hint, done during unshard) and stacks the 4 batches.  This removes
the K/V-projection redundancy a query-split sharding would have.

Per-core dataflow (bf16 matmul operands, fp32 PSUM):
  proj     QT/KT[hd,2048] per head-pair frame (head 2p on partitions 0-63,
           head 2p+1 on 64-127), V_aug[tok,head,65] token-major (col 64 =
           ones -> softmax denominators ride along row 64 of the attnV
           accumulator).  X^T for K/V kept SBUF-resident, xq streamed.
  scores   per head K=64 matmuls (lhsT = KT head rows, rhs = QT head rows,
           no zero padding), [128 s, 1024 q] PSUM tile per head.
  softmax  one exp per head per s-block [128,1024] on ScalarE (the
           bottleneck engine: ~1.33us each, 256 total).
  attnV    M=65 matmuls accumulate OP[65,1024] per head over 16 s-blocks.
  norm     OP rows 0-63 copied unnormalized to HT (bf16); reciprocal of
           row 64 on DVE, broadcast across partitions via DRAM bounce,
           one in-place DVE multiply per query group.
  out-proj out[tok,dm] partial = HT^T(lhsT) @ Wout, streamed to DRAM.
Projection matmul groups for pairs 1-3 are emitted at query-group
boundaries where PSUM tiles are free, filling PE time while ScalarE
drains its exp backlog.
"""

import os
import sys

for _p in ("/opt/trn_rl_repo", "/opt/pypackages"):
    if _p not in sys.path:
        sys.path.append(_p)

import numpy as np

H_TOT, D, DK, DV = 16, 1024, 64, 64
B, L = 4, 2048
H = 8  # heads per core
NP = 4  # head pairs per core
P = 128
NKB = D // P  # 8 contraction blocks over d_model
NSB = L // P  # 16 key-token blocks
NQG = 2  # query groups of 1024

_CACHE = {}


def _build_bass():
    import concourse.bass as bass
    import concourse.tile as tile
    from concourse import mybir
    from concourse.bass import ts

    f32 = mybir.dt.float32
    bf16 = mybir.dt.bfloat16
    EXP = mybir.ActivationFunctionType.Exp

    nc = bass.Bass()
    xqt = nc.dram_tensor("xqt", [D, L], bf16, kind="ExternalInput")
    xkt = nc.dram_tensor("xkt", [D, L], bf16, kind="ExternalInput")
    xvt = nc.dram_tensor("xvt", [D, L], bf16, kind="ExternalInput")
    wq = nc.dram_tensor("wq", [NKB, P, NP, P], bf16, kind="ExternalInput")
    wk = nc.dram_tensor("wk", [NKB, P, NP, P], bf16, kind="ExternalInput")
    wv = nc.dram_tensor("wv", [D, H * DV], bf16, kind="ExternalInput")
    wout = nc.dram_tensor("wout", [H * DV, D], bf16, kind="ExternalInput")
    out = nc.dram_tensor("out", [L, D], f32, kind="ExternalOutput")

    lp = nc.allow_low_precision(
        reason="bf16 matmul operands; accumulation stays fp32 in PSUM"
    )
    lp.__enter__()
    with tile.TileContext(nc) as tc:
        with (
            tc.tile_pool(name="persist", bufs=1) as persist,
            tc.tile_pool(name="xin", bufs=3) as xin,
            tc.tile_pool(name="aep", bufs=3) as aep,
            tc.tile_pool(name="recp", bufs=2) as recp,
            tc.tile_pool(name="bcp", bufs=2) as bcp,
            tc.tile_pool(name="outp", bufs=2) as outp,
            tc.tile_pool(name="dramp", bufs=2, space="DRAM") as dramp,
            tc.tile_pool(name="ps", bufs=1, space="PSUM") as ps,
        ):
            # ---- persistent SBUF ----
            XK = persist.tile([P, NKB, L], bf16)  # 32 KB/part
            XV = persist.tile([P, NKB, L], bf16)  # 32 KB/part
            QT = persist.tile([P, NP, L], bf16)  # 16 KB/part
            KT = persist.tile([P, NP, L], bf16)  # 16 KB/part
            VA = persist.tile([P, NSB, H, DV + 1], bf16)  # 16.25 KB/part
            HT = persist.tile([P, NP, L], bf16)  # 16 KB/part
            WQ = persist.tile([P, NKB, NP, P], bf16)  # 8 KB/part
            WK = persist.tile([P, NKB, NP, P], bf16)  # 8 KB/part
            WV = persist.tile([P, NKB, H * DV], bf16)  # 8 KB/part
            WO = persist.tile([P, NP, D], bf16)  # 8 KB/part

            # ---- prologue DMAs ----
            for k in range(NKB):
                nc.sync.dma_start(out=WQ[:, k], in_=wq[k])
                nc.sync.dma_start(out=WK[:, k], in_=wk[k])
                nc.sync.dma_start(out=WV[:, k], in_=wv[ts(k, P), :])
            for k in range(NP):
                nc.sync.dma_start(out=WO[:, k], in_=wout[ts(k, P), :])
            nc.gpsimd.memset(VA[:, :, :, DV : DV + 1], 1.0)
            for k in range(NKB):
                (nc.sync if k % 2 == 0 else nc.gpsimd).dma_start(
                    out=XK[:, k], in_=xkt[ts(k, P), :]
                )
                (nc.gpsimd if k % 2 == 0 else nc.sync).dma_start(
                    out=XV[:, k], in_=xvt[ts(k, P), :]
                )

            # ---- projection group emitters ----
            def emit_q_proj_group(p, g):
                pt = ps.tile([P, 1024], f32, tag="sp", bufs=2, name=f"qp_{p}_{g}")
                for k in range(NKB):
                    xt = xin.tile([P, 1024], bf16, tag="xq")
                    nc.gpsimd.dma_start(
                        out=xt, in_=xqt[ts(k, P), g * 1024 : g * 1024 + 1024]
                    )
                    for qn in range(2):
                        nc.tensor.matmul(
                            pt[:, qn * 512 : qn * 512 + 512],
                            lhsT=WQ[:, k, p, :],
                            rhs=xt[:, qn * 512 : qn * 512 + 512],
                            start=(k == 0),
                            stop=(k == NKB - 1),
                        )
                nc.vector.tensor_copy(QT[:, p, g * 1024 : g * 1024 + 1024], pt)

            def emit_k_proj_group(p, g):
                pt = ps.tile([P, 1024], f32, tag="sp", bufs=2, name=f"kp_{p}_{g}")
                for k in range(NKB):
                    for qn in range(2):
                        c0 = g * 1024 + qn * 512
                        nc.tensor.matmul(
                            pt[:, qn * 512 : qn * 512 + 512],
                            lhsT=WK[:, k, p, :],
                            rhs=XK[:, k, c0 : c0 + 512],
                            start=(k == 0),
                            stop=(k == NKB - 1),
                        )
                nc.vector.tensor_copy(KT[:, p, g * 1024 : g * 1024 + 1024], pt)

            def emit_v_proj_group(j):
                # token blocks 2j, 2j+1 -> VA
                pt = ps.tile([P, 1024], f32, tag="sp", bufs=2, name=f"vp_{j}")
                for k in range(NKB):
                    for jj in range(2):
                        nc.tensor.matmul(
                            pt[:, jj * 512 : jj * 512 + 512],
                            lhsT=XV[:, k, ts(2 * j + jj, P)],
                            rhs=WV[:, k, :],
                            start=(k == 0),
                            stop=(k == NKB - 1),
                        )
                for jj in range(2):
                    nc.vector.tensor_copy(
                        VA[:, 2 * j + jj, :, 0:DV],
                        pt[:, jj * 512 : jj * 512 + 512].rearrange(
                            "p (h v) -> p h v", h=H
                        ),
                    )

            # ---- prologue compute: pair 0 Q/K, all V blocks ----
            for g in range(NQG):
                emit_q_proj_group(0, g)
            for g in range(NQG):
                emit_k_proj_group(0, g)
            for j in range(8):
                emit_v_proj_group(j)

            # proj groups deferred to qg boundaries: 2 per boundary
            boundary_work = []
            for p in range(1, NP):
                for g in range(NQG):
                    boundary_work.append(lambda p=p, g=g: emit_k_proj_group(p, g))
                for g in range(NQG):
                    boundary_work.append(lambda p=p, g=g: emit_q_proj_group(p, g))
            bw_i = 0

            # ---- attention ----
            for p in range(NP):
                hA, hB = 2 * p, 2 * p + 1
                for qg in range(NQG):
                    opa = ps.tile(
                        [DV + 1, 1024], f32, tag="opa", bufs=1, name=f"opa_{p}_{qg}"
                    )
                    opb = ps.tile(
                        [DV + 1, 1024], f32, tag="opb", bufs=1, name=f"opb_{p}_{qg}"
                    )
                    def emit_attnv(s, aea, aeb):
                        for qn in range(2):
                            nc.tensor.matmul(
                                opa[:, qn * 512 : qn * 512 + 512],
                                lhsT=VA[:, s, hA, :],
                                rhs=aea[:, qn * 512 : qn * 512 + 512],
                                start=(s == 0),
                                stop=(s == NSB - 1),
                            )
                            nc.tensor.matmul(
                                opb[:, qn * 512 : qn * 512 + 512],
                                lhsT=VA[:, s, hB, :],
                                rhs=aeb[:, qn * 512 : qn * 512 + 512],
                                start=(s == 0),
                                stop=(s == NSB - 1),
                            )

                    prev = None
                    for s in range(NSB):
                        sa = ps.tile([P, 1024], f32, tag="sp", bufs=2)
                        sb = ps.tile([P, 1024], f32, tag="sp", bufs=2)
                        for qn in range(2):
                            c0 = qg * 1024 + qn * 512
                            nc.tensor.matmul(
                                sa[:, qn * 512 : qn * 512 + 512],
                                lhsT=KT[0:64, p, ts(s, P)],
                                rhs=QT[0:64, p, c0 : c0 + 512],
                                start=True,
                                stop=True,
                            )
                            nc.tensor.matmul(
                                sb[:, qn * 512 : qn * 512 + 512],
                                lhsT=KT[64:128, p, ts(s, P)],
                                rhs=QT[64:128, p, c0 : c0 + 512],
                                start=True,
                                stop=True,
                            )
                        aea = aep.tile([P, 1024], bf16, tag="ae")
                        aeb = aep.tile([P, 1024], bf16, tag="ae")
                        nc.scalar.activation(out=aea, in_=sa, func=EXP, scale=0.125)
                        nc.scalar.activation(out=aeb, in_=sb, func=EXP, scale=0.125)
                        # software pipeline: attnV for the PREVIOUS s-block,
                        # whose exp outputs are ready -> the in-order PE queue
                        # never blocks on the current exp.
                        if prev is not None:
                            emit_attnv(*prev)
                        prev = (s, aea, aeb)
                    emit_attnv(*prev)

                    # ---- qg epilogue ----
                    cols = slice(qg * 1024, qg * 1024 + 1024)
                    # unnormalized heads + denominator rows out of PSUM fast
                    # (frees op tiles; the slow reciprocal reads SBUF copies)
                    da = recp.tile([1, 1024], f32, tag="den")
                    db = recp.tile([1, 1024], f32, tag="den")
                    nc.vector.tensor_copy(da[:, :], opa[DV : DV + 1, :])
                    nc.vector.tensor_copy(db[:, :], opb[DV : DV + 1, :])
                    nc.vector.tensor_copy(HT[0:64, p, cols], opa[0:DV, :])
                    nc.vector.tensor_copy(HT[64:128, p, cols], opb[0:DV, :])
                    ra = recp.tile([1, 1024], f32, tag="rec")
                    rb = recp.tile([1, 1024], f32, tag="rec")
                    nc.vector.reciprocal(ra[:, :], da[:, :])
                    nc.vector.reciprocal(rb[:, :], db[:, :])
                    r16 = recp.tile([33, 1024], bf16, tag="rec16")
                    nc.vector.tensor_copy(r16[0:1, :], ra[:, :])
                    nc.vector.tensor_copy(r16[32:33, :], rb[:, :])
                    rcb = dramp.tile([2, 1024], bf16, tag="rcb", name=f"rcb_{p}_{qg}")
                    nc.sync.dma_start(out=rcb[0:1, :], in_=r16[0:1, :])
                    nc.sync.dma_start(out=rcb[1:2, :], in_=r16[32:33, :])
                    bc = bcp.tile([P, 1024], bf16, tag="bc")
                    nc.gpsimd.dma_start(
                        out=bc[0:64, :], in_=rcb[0:1, :].to_broadcast((64, 1024))
                    )
                    nc.sync.dma_start(
                        out=bc[64:128, :], in_=rcb[1:2, :].to_broadcast((64, 1024))
                    )
                    nc.vector.tensor_mul(HT[:, p, cols], HT[:, p, cols], bc[:, :])
                    # boundary projection work (PE fills while ScalarE drains)
                    for _ in range(2):
                        if bw_i < len(boundary_work):
                            boundary_work[bw_i]()
                            bw_i += 1

            while bw_i < len(boundary_work):
                boundary_work[bw_i]()
                bw_i += 1

            # ---- out-projection (partial over this core's heads) ----
            for m in range(NSB):
                pt = ps.tile([P, 1024], f32, tag="sp", bufs=2, name=f"po_{m}")
                for dh in range(2):
                    for kp in range(NP):
                        nc.tensor.matmul(
                            pt[:, dh * 512 : dh * 512 + 512],
                            lhsT=HT[:, kp, ts(m, P)],
                            rhs=WO[:, kp, dh * 512 : dh * 512 + 512],
                            start=(kp == 0),
                            stop=(kp == NP - 1),
                        )
                ot = outp.tile([P, 1024], f32, tag="ot", name=f"ot_{m}")
                nc.vector.tensor_copy(ot, pt)
                (nc.gpsimd if m % 2 == 0 else nc.sync).dma_start(
                    out=out[ts(m, P), :], in_=ot
                )
    lp.__exit__(None, None, None)

    _split_multi_waits(nc)
    return nc


def _split_multi_waits(nc, max_waits: int = 1):
    """Walrus's setupSyncWait rejects instructions carrying more than a
    struct-specific number of sync waits (e.g. the Tile kernel-tail Drain
    gathers one wait per live semaphore). Hoist excess waits into prepended
    single-wait NoOps on the same engine."""
    from concourse import mybir

    for f in nc.m.functions:
        for blk in f.blocks:
            out = []
            for inst in blk.instructions:
                si = inst.sync_info
                waits = list(si.on_wait) if (si is not None and si.on_wait) else []
                if len(waits) > max_waits:
                    keep = waits[-max_waits:]
                    for w in waits[:-max_waits]:
                        nop = mybir.InstNoOp(
                            name=nc.get_next_instruction_name(),
                            ins=[],
                            outs=[],
                            sync_info=mybir.SyncInfo(on_wait=[w], on_update=[]),
                        )
                        nop.engine = inst.engine
                        try:
                            nop.bass_nofuse = True
                        except Exception:
                            pass
                        nc.register_instruction(nop)
                        out.append(nop)
                    si.on_wait = keep
                out.append(inst)
            blk.instructions = out


def _get_nc():
    if "nc" not in _CACHE:
        _CACHE["nc"] = _build_bass()
    return _CACHE["nc"]


def _prep_in_maps(x_query, x_key, x_value, Wq, Wk, Wv, Wout):
    import ml_dtypes

    bf = ml_dtypes.bfloat16
    x_query = np.asarray(x_query, dtype=np.float32)
    x_key = np.asarray(x_key, dtype=np.float32)
    x_value = np.asarray(x_value, dtype=np.float32)
    Wq = np.asarray(Wq, np.float32)
    Wk = np.asarray(Wk, np.float32)
    Wv = np.asarray(Wv, np.float32)
    Wout = np.asarray(Wout, np.float32)

    # per-batch transposed activations (shared by the 2 cores of a batch)
    xqT = [np.ascontiguousarray(x_query[b].T).astype(bf) for b in range(B)]
    xkT = [np.ascontiguousarray(x_key[b].T).astype(bf) for b in range(B)]
    xvT = [np.ascontiguousarray(x_value[b].T).astype(bf) for b in range(B)]

    # per head-half weight slices
    wq_h, wk_h, wv_h, wo_h = [], [], [], []
    for hh in range(2):
        hs = slice(hh * H, hh * H + H)
        wq_cat = Wq[hs].transpose(1, 0, 2).reshape(D, H * DK)
        wk_cat = Wk[hs].transpose(1, 0, 2).reshape(D, H * DK)
        wq_h.append(np.ascontiguousarray(wq_cat.reshape(NKB, P, NP, P)).astype(bf))
        wk_h.append(np.ascontiguousarray(wk_cat.reshape(NKB, P, NP, P)).astype(bf))
        wv_h.append(
            np.ascontiguousarray(
                Wv[hs].transpose(1, 0, 2).reshape(D, H * DV)
            ).astype(bf)
        )
        wo_h.append(
            np.ascontiguousarray(Wout[hh * H * DV : (hh + 1) * H * DV, :]).astype(bf)
        )

    in_maps = []
    for c in range(8):
        b, hh = divmod(c, 2)
        in_maps.append(
            {
                "xqt": xqT[b],
                "xkt": xkT[b],
                "xvt": xvT[b],
                "wq": wq_h[hh],
                "wk": wk_h[hh],
                "wv": wv_h[hh],
                "wout": wo_h[hh],
            }
        )
    return in_maps


def kernel(x_query, x_key, x_value, Wq, Wk, Wv, Wout):
    from concourse.bass_utils import run_bass_kernel_spmd

    nc = _get_nc()
    in_maps = _prep_in_maps(x_query, x_key, x_value, Wq, Wk, Wv, Wout)
    trace = bool(int(os.environ.get("MHA_TRACE", "0")))
    res = run_bass_kernel_spmd(nc, in_maps, list(range(8)), trace=trace)
    _CACHE["last_result"] = res
    out = np.empty((B, L, D), np.float32)
    for b in range(B):
        out[b] = res.results[2 * b]["out"] + res.results[2 * b + 1]["out"]
    return out


# revision 11
# speedup vs baseline: 1.1657x; 1.0094x over previous
"""Multi-head attention Trainium2 Bass kernel (nn_MultiHeadAttention_69655779607087).

Problem (hardcoded): B=4, L=2048, D_MODEL=1024, H=16, D_QK=D_V=64, fp32.

Sharding (8 cores, tensor-parallel heads x batch, host-side pair reduce):
core c handles batch b=c//2 and head-half hh=c%2 (8 heads), producing a
PARTIAL output [2048, 1024] (its heads' contribution to the out-projection).
The host sums the two partial outputs per batch (the "all-reduce" of the
sharding hint, done during unshard) and stacks the 4 batches.  This removes
the K/V-projection redundancy a query-split sharding would have.

Per-core dataflow (bf16 matmul operands, fp32 PSUM):
  proj     QT/KT[hd,2048] per head-pair frame (head 2p on partitions 0-63,
           head 2p+1 on 64-127), V_aug[tok,head,65] token-major (col 64 =
           ones -> softmax denominators ride along row 64 of the attnV
           accumulator).  X^T for K/V kept SBUF-resident, xq streamed.
  scores   per head K=64 matmuls (lhsT = KT head rows, rhs = QT head rows,
           no zero padding), [128 s, 1024 q] PSUM tile per head.
  softmax  one exp per head per s-block [128,1024] on ScalarE (the
           bottleneck engine: ~1.33us each, 256 total).
  attnV    M=65 matmuls accumulate OP[65,1024] per head over 16 s-blocks.
  norm     OP rows 0-63 copied unnormalized to HT (bf16); reciprocal of
           row 64 on DVE, broadcast across partitions via DRAM bounce,
           one in-place DVE multiply per query group.
  out-proj out[tok,dm] partial = HT^T(lhsT) @ Wout, streamed to DRAM.
Projection matmul groups for pairs 1-3 are emitted at query-group
boundaries where PSUM tiles are free, filling PE time while ScalarE
drains its exp backlog.
"""

import os
import sys

for _p in ("/opt/trn_rl_repo", "/opt/pypackages"):
    if _p not in sys.path:
        sys.path.append(_p)

import numpy as np

H_TOT, D, DK, DV = 16, 1024, 64, 64
B, L = 4, 2048
H = 8  # heads per core
NP = 4  # head pairs per core
P = 128
NKB = D // P  # 8 contraction blocks over d_model
NSB = L // P  # 16 key-token blocks
NQG = 2  # query groups of 1024

_CACHE = {}


def _build_bass():
    import concourse.bass as bass
    import concourse.tile as tile
    from concourse import mybir
    from concourse.bass import ts

    f32 = mybir.dt.float32
    bf16 = mybir.dt.bfloat16
    EXP = mybir.ActivationFunctionType.Exp

    nc = bass.Bass()
    xqt = nc.dram_tensor("xqt", [D, L], bf16, kind="ExternalInput")
    xkt = nc.dram_tensor("xkt", [D, L], bf16, kind="ExternalInput")
    xvt = nc.dram_tensor("xvt", [D, L], bf16, kind="ExternalInput")
    wq = nc.dram_tensor("wq", [NKB, P, NP, P], bf16, kind="ExternalInput")
    wk = nc.dram_tensor("wk", [NKB, P, NP, P], bf16, kind="ExternalInput")
    wv = nc.dram_tensor("wv", [D, H * DV], bf16, kind="ExternalInput")
    wout = nc.dram_tensor("wout", [H * DV, D], bf16, kind="ExternalInput")
    out = nc.dram_tensor("out", [L, D], f32, kind="ExternalOutput")

    lp = nc.allow_low_precision(
        reason="bf16 matmul operands; accumulation stays fp32 in PSUM"
    )
    lp.__enter__()
    with tile.TileContext(nc) as tc:
        with (
            tc.tile_pool(name="persist", bufs=1) as persist,
            tc.tile_pool(name="xin", bufs=3) as xin,
            tc.tile_pool(name="aep", bufs=4) as aep,
            tc.tile_pool(name="recp", bufs=2) as recp,
            tc.tile_pool(name="bcp", bufs=2) as bcp,
            tc.tile_pool(name="outp", bufs=2) as outp,
            tc.tile_pool(name="dramp", bufs=2, space="DRAM") as dramp,
            tc.tile_pool(name="ps", bufs=1, space="PSUM") as ps,
        ):
            # ---- persistent SBUF ----
            XK = persist.tile([P, NKB, L], bf16)  # 32 KB/part
            XV = persist.tile([P, NKB, L], bf16)  # 32 KB/part
            QT = persist.tile([P, NP, L], bf16)  # 16 KB/part
            KT = persist.tile([P, NP, L], bf16)  # 16 KB/part
            VA = persist.tile([P, NSB, H, DV + 1], bf16)  # 16.25 KB/part
            HT = persist.tile([P, NP, L], bf16)  # 16 KB/part
            WQ = persist.tile([P, NKB, NP, P], bf16)  # 8 KB/part
            WK = persist.tile([P, NKB, NP, P], bf16)  # 8 KB/part
            WV = persist.tile([P, NKB, H * DV], bf16)  # 8 KB/part
            WO = persist.tile([P, NP, D], bf16)  # 8 KB/part

            # ---- prologue DMAs ----
            for k in range(NKB):
                nc.sync.dma_start(out=WQ[:, k], in_=wq[k])
                nc.sync.dma_start(out=WK[:, k], in_=wk[k])
                nc.sync.dma_start(out=WV[:, k], in_=wv[ts(k, P), :])
            for k in range(NP):
                nc.sync.dma_start(out=WO[:, k], in_=wout[ts(k, P), :])
            nc.gpsimd.memset(VA[:, :, :, DV : DV + 1], 1.0)
            for k in range(NKB):
                (nc.sync if k % 2 == 0 else nc.gpsimd).dma_start(
                    out=XK[:, k], in_=xkt[ts(k, P), :]
                )
                (nc.gpsimd if k % 2 == 0 else nc.sync).dma_start(
                    out=XV[:, k], in_=xvt[ts(k, P), :]
                )

            # ---- projection group emitters ----
            def emit_q_proj_group(p, g):
                pt = ps.tile([P, 1024], f32, tag="sp", bufs=2, name=f"qp_{p}_{g}")
                for k in range(NKB):
                    xt = xin.tile([P, 1024], bf16, tag="xq")
                    nc.gpsimd.dma_start(
                        out=xt, in_=xqt[ts(k, P), g * 1024 : g * 1024 + 1024]
                    )
                    for qn in range(2):
                        nc.tensor.matmul(
                            pt[:, qn * 512 : qn * 512 + 512],
                            lhsT=WQ[:, k, p, :],
                            rhs=xt[:, qn * 512 : qn * 512 + 512],
                            start=(k == 0),
                            stop=(k == NKB - 1),
                        )
                nc.vector.tensor_copy(QT[:, p, g * 1024 : g * 1024 + 1024], pt)

            def emit_k_proj_group(p, g):
                pt = ps.tile([P, 1024], f32, tag="sp", bufs=2, name=f"kp_{p}_{g}")
                for k in range(NKB):
                    for qn in range(2):
                        c0 = g * 1024 + qn * 512
                        nc.tensor.matmul(
                            pt[:, qn * 512 : qn * 512 + 512],
                            lhsT=WK[:, k, p, :],
                            rhs=XK[:, k, c0 : c0 + 512],
                            start=(k == 0),
                            stop=(k == NKB - 1),
                        )
                nc.vector.tensor_copy(KT[:, p, g * 1024 : g * 1024 + 1024], pt)

            def emit_v_proj_group(j):
                # token blocks 2j, 2j+1 -> VA
                pt = ps.tile([P, 1024], f32, tag="sp", bufs=2, name=f"vp_{j}")
                for k in range(NKB):
                    for jj in range(2):
                        nc.tensor.matmul(
                            pt[:, jj * 512 : jj * 512 + 512],
                            lhsT=XV[:, k, ts(2 * j + jj, P)],
                            rhs=WV[:, k, :],
                            start=(k == 0),
                            stop=(k == NKB - 1),
                        )
                for jj in range(2):
                    nc.vector.tensor_copy(
                        VA[:, 2 * j + jj, :, 0:DV],
                        pt[:, jj * 512 : jj * 512 + 512].rearrange(
                            "p (h v) -> p h v", h=H
                        ),
                    )

            # ---- prologue compute: pair 0 Q/K, all V blocks ----
            for g in range(NQG):
                emit_q_proj_group(0, g)
            for g in range(NQG):
                emit_k_proj_group(0, g)
            for j in range(8):
                emit_v_proj_group(j)

            # proj groups deferred to qg boundaries: 2 per boundary
            boundary_work = []
            for p in range(1, NP):
                for g in range(NQG):
                    boundary_work.append(lambda p=p, g=g: emit_k_proj_group(p, g))
                for g in range(NQG):
                    boundary_work.append(lambda p=p, g=g: emit_q_proj_group(p, g))
            bw_i = 0

            # ---- attention ----
            for p in range(NP):
                hA, hB = 2 * p, 2 * p + 1
                for qg in range(NQG):
                    opa = ps.tile(
                        [DV + 1, 1024], f32, tag="opa", bufs=1, name=f"opa_{p}_{qg}"
                    )
                    opb = ps.tile(
                        [DV + 1, 1024], f32, tag="opb", bufs=1, name=f"opb_{p}_{qg}"
                    )
                    def emit_attnv(s, aea, aeb):
                        for qn in range(2):
                            nc.tensor.matmul(
                                opa[:, qn * 512 : qn * 512 + 512],
                                lhsT=VA[:, s, hA, :],
                                rhs=aea[:, qn * 512 : qn * 512 + 512],
                                start=(s == 0),
                                stop=(s == NSB - 1),
                            )
                            nc.tensor.matmul(
                                opb[:, qn * 512 : qn * 512 + 512],
                                lhsT=VA[:, s, hB, :],
                                rhs=aeb[:, qn * 512 : qn * 512 + 512],
                                start=(s == 0),
                                stop=(s == NSB - 1),
                            )

                    prev = None
                    for s in range(NSB):
                        sa = ps.tile([P, 1024], f32, tag="sp", bufs=2)
                        sb = ps.tile([P, 1024], f32, tag="sp", bufs=2)
                        for qn in range(2):
                            c0 = qg * 1024 + qn * 512
                            nc.tensor.matmul(
                                sa[:, qn * 512 : qn * 512 + 512],
                                lhsT=KT[0:64, p, ts(s, P)],
                                rhs=QT[0:64, p, c0 : c0 + 512],
                                start=True,
                                stop=True,
                            )
                            nc.tensor.matmul(
                                sb[:, qn * 512 : qn * 512 + 512],
                                lhsT=KT[64:128, p, ts(s, P)],
                                rhs=QT[64:128, p, c0 : c0 + 512],
                                start=True,
                                stop=True,
                            )
                        aea = aep.tile([P, 1024], bf16, tag="ae")
                        aeb = aep.tile([P, 1024], bf16, tag="ae")
                        nc.scalar.activation(out=aea, in_=sa, func=EXP, scale=0.125)
                        nc.scalar.activation(out=aeb, in_=sb, func=EXP, scale=0.125)
                        # software pipeline: attnV for the PREVIOUS s-block,
                        # whose exp outputs are ready -> the in-order PE queue
                        # never blocks on the current exp.
                        if prev is not None:
                            emit_attnv(*prev)
                        prev = (s, aea, aeb)
                    emit_attnv(*prev)

                    # ---- qg epilogue ----
                    cols = slice(qg * 1024, qg * 1024 + 1024)
                    # unnormalized heads + denominator rows out of PSUM fast
                    # (frees op tiles; the slow reciprocal reads SBUF copies)
                    da = recp.tile([1, 1024], f32, tag="den")
                    db = recp.tile([1, 1024], f32, tag="den")
                    nc.vector.tensor_copy(da[:, :], opa[DV : DV + 1, :])
                    nc.vector.tensor_copy(db[:, :], opb[DV : DV + 1, :])
                    nc.vector.tensor_copy(HT[0:64, p, cols], opa[0:DV, :])
                    nc.vector.tensor_copy(HT[64:128, p, cols], opb[0:DV, :])
                    ra = recp.tile([1, 1024], f32, tag="rec")
                    rb = recp.tile([1, 1024], f32, tag="rec")
                    nc.vector.reciprocal(ra[:, :], da[:, :])
                    nc.vector.reciprocal(rb[:, :], db[:, :])
                    r16 = recp.tile([33, 1024], bf16, tag="rec16")
                    nc.vector.tensor_copy(r16[0:1, :], ra[:, :])
                    nc.vector.tensor_copy(r16[32:33, :], rb[:, :])
                    rcb = dramp.tile([2, 1024], bf16, tag="rcb", name=f"rcb_{p}_{qg}")
                    nc.sync.dma_start(out=rcb[0:1, :], in_=r16[0:1, :])
                    nc.sync.dma_start(out=rcb[1:2, :], in_=r16[32:33, :])
                    bc = bcp.tile([P, 1024], bf16, tag="bc")
                    nc.gpsimd.dma_start(
                        out=bc[0:64, :], in_=rcb[0:1, :].to_broadcast((64, 1024))
                    )
                    nc.sync.dma_start(
                        out=bc[64:128, :], in_=rcb[1:2, :].to_broadcast((64, 1024))
                    )
                    nc.vector.tensor_mul(HT[:, p, cols], HT[:, p, cols], bc[:, :])
                    # boundary projection work (PE fills while ScalarE drains)
                    for _ in range(2):
                        if bw_i < len(boundary_work):
                            boundary_work[bw_i]()
                            bw_i += 1

            while bw_i < len(boundary_work):
                boundary_work[bw_i]()
                bw_i += 1

            # ---- out-projection (partial over this core's heads) ----
            for m in range(NSB):
                pt = ps.tile([P, 1024], f32, tag="sp", bufs=2, name=f"po_{m}")
                for dh in range(2):
                    for kp in range(NP):
                        nc.tensor.matmul(
                            pt[:, dh * 512 : dh * 512 + 512],
                            lhsT=HT[:, kp, ts(m, P)],
                            rhs=WO[:, kp, dh * 512 : dh * 512 + 512],
                            start=(kp == 0),
                            stop=(kp == NP - 1),
                        )
                ot = outp.tile([P, 1024], f32, tag="ot", name=f"ot_{m}")
                nc.vector.tensor_copy(ot, pt)
                (nc.gpsimd if m % 2 == 0 else nc.sync).dma_start(
                    out=out[ts(m, P), :], in_=ot
                )
    lp.__exit__(None, None, None)

    _split_multi_waits(nc)
    return nc


def _split_multi_waits(nc, max_waits: int = 1):
    """Walrus's setupSyncWait rejects instructions carrying more than a
    struct-specific number of sync waits (e.g. the Tile kernel-tail Drain
    gathers one wait per live semaphore). Hoist excess waits into prepended
    single-wait NoOps on the same engine."""
    from concourse import mybir

    for f in nc.m.functions:
        for blk in f.blocks:
            out = []
            for inst in blk.instructions:
                si = inst.sync_info
                waits = list(si.on_wait) if (si is not None and si.on_wait) else []
                if len(waits) > max_waits:
                    keep = waits[-max_waits:]
                    for w in waits[:-max_waits]:
                        nop = mybir.InstNoOp(
                            name=nc.get_next_instruction_name(),
                            ins=[],
                            outs=[],
                            sync_info=mybir.SyncInfo(on_wait=[w], on_update=[]),
                        )
                        nop.engine = inst.engine
                        try:
                            nop.bass_nofuse = True
                        except Exception:
                            pass
                        nc.register_instruction(nop)
                        out.append(nop)
                    si.on_wait = keep
                out.append(inst)
            blk.instructions = out


def _get_nc():
    if "nc" not in _CACHE:
        _CACHE["nc"] = _build_bass()
    return _CACHE["nc"]


def _prep_in_maps(x_query, x_key, x_value, Wq, Wk, Wv, Wout):
    import ml_dtypes

    bf = ml_dtypes.bfloat16
    x_query = np.asarray(x_query, dtype=np.float32)
    x_key = np.asarray(x_key, dtype=np.float32)
    x_value = np.asarray(x_value, dtype=np.float32)
    Wq = np.asarray(Wq, np.float32)
    Wk = np.asarray(Wk, np.float32)
    Wv = np.asarray(Wv, np.float32)
    Wout = np.asarray(Wout, np.float32)

    # per-batch transposed activations (shared by the 2 cores of a batch)
    xqT = [np.ascontiguousarray(x_query[b].T).astype(bf) for b in range(B)]
    xkT = [np.ascontiguousarray(x_key[b].T).astype(bf) for b in range(B)]
    xvT = [np.ascontiguousarray(x_value[b].T).astype(bf) for b in range(B)]

    # per head-half weight slices
    wq_h, wk_h, wv_h, wo_h = [], [], [], []
    for hh in range(2):
        hs = slice(hh * H, hh * H + H)
        wq_cat = Wq[hs].transpose(1, 0, 2).reshape(D, H * DK)
        wk_cat = Wk[hs].transpose(1, 0, 2).reshape(D, H * DK)
        wq_h.append(np.ascontiguousarray(wq_cat.reshape(NKB, P, NP, P)).astype(bf))
        wk_h.append(np.ascontiguousarray(wk_cat.reshape(NKB, P, NP, P)).astype(bf))
        wv_h.append(
            np.ascontiguousarray(
                Wv[hs].transpose(1, 0, 2).reshape(D, H * DV)
            ).astype(bf)
        )
        wo_h.append(
            np.ascontiguousarray(Wout[hh * H * DV : (hh + 1) * H * DV, :]).astype(bf)
        )

    in_maps = []
    for c in range(8):
        b, hh = divmod(c, 2)
        in_maps.append(
            {
                "xqt": xqT[b],
                "xkt": xkT[b],
                "xvt": xvT[b],
                "wq": wq_h[hh],
                "wk": wk_h[hh],
                "wv": wv_h[hh],
                "wout": wo_h[hh],
            }
        )
    return in_maps


def kernel(x_query, x_key, x_value, Wq, Wk, Wv, Wout):
    from concourse.bass_utils import run_bass_kernel_spmd

    nc = _get_nc()
    in_maps = _prep_in_maps(x_query, x_key, x_value, Wq, Wk, Wv, Wout)
    trace = bool(int(os.environ.get("MHA_TRACE", "0")))
    res = run_bass_kernel_spmd(nc, in_maps, list(range(8)), trace=trace)
    _CACHE["last_result"] = res
    out = np.empty((B, L, D), np.float32)
    for b in range(B):
        out[b] = res.results[2 * b]["out"] + res.results[2 * b + 1]["out"]
    return out
